# revision 45
# baseline (speedup 1.0000x reference)
"""Trainium2 Bass kernel for nn_CINComp: out[b,o,d] = sum_{i,j} W[o,i*64+j]*feature[b,i,d]*base[b,j,d] + bias[o].

Sharding: data-parallel over batch B=1024 across 8 cores (128 batches/core).

Per-core algorithm (v5, streaming):
  - contraction dim ij = 4096 split into 32 K-chunks of 128 = (2 i-rows x 64 j)
  - gt2[p=(dup,j), (b,d)] holds G transposed + duplicated (host-prepped, bf16)
  - fbc[p, g, c, n] holds the F-row broadcast (f[2c+dup] replicated across the
    64 j-partitions) PRE-BUILT ON HOST in bf16 and streamed from HBM, so the
    DVE multiply is an all-SBUF bf16 tensor_tensor -> 2x_1P mode (the
    original PE-selector baseline left it in PSUM fp32, capping DVE at 1x =
    its bottleneck); the stream is triple-buffered and segment-granular so
    DMA, DVE and PE pipeline with no barriers
  - PE contracts W^T-chunk @ P into PSUM acc (bf16 matmuls, FWL fast path,
    LDWEIGHTS hidden inside the MM stream); acc uses all 8 PSUM banks (one
    per group) so no accumulator is ever reused -> no WAR waits anywhere
  - ACT (otherwise idle) adds bias during the PSUM->SBUF output copy
    (Identity with per-partition bias AP) and issues the output DMAs

The end-to-end time is Sync-DMA-ring bound (~35.5 MB at ~400 GB/s), so the
ring carries gt2/wt before the fbc stream in FIFO order and everything else
rides the scalar-HWDGE ring.

Sync discipline: fused LDWEIGHTS+MATMUL, TT, ACT and DMA instruction structs
accept only ONE semaphore wait.  Every DMA-landed tile is "touched" with a
1-element self-copy on the engine whose FIFO must carry the dependency (DVE
for gt2/wt/fbc, ACT for bias); _strip_self_waits then drops all transitively-
implied waits.  Only REAL data deps order instructions (the Tile scheduler
reorders freely otherwise).
"""

import numpy as np

import concourse.bass as bass
import concourse.mybir as mybir
import concourse.tile as tile
from concourse.bass import ts
from concourse.bass_utils import run_bass_kernel_spmd

B, HK, H0, D, O = 1024, 64, 64, 32, 128
NCORES = 8
BLOC = B // NCORES          # 128 batches per core
GROUPS = 8                  # batch groups per core
GB = BLOC // GROUPS         # 16 batches per group
N = GB * D                  # 512 = matmul free dim per group
NCHUNK = 32                 # K chunks of 128 over ij=4096
F32 = mybir.dt.float32
BF16 = mybir.dt.bfloat16

_CACHE = {}


def _strip_self_waits(nc: bass.Bass) -> None:
    """Transitively-minimal semaphore waits (see module docstring)."""
    UPD = ("sem-inc", "sem-add-imm")
    insts = [i for bb in nc.m.functions[0].blocks for i in bb.instructions]

    bad_sems = set()
    for i in insts:
        si = getattr(i, "sync_info", None)
        if si is None:
            continue
        for u in si.on_update:
            if u.sync_type != "semaphore" or u.update_mode not in UPD:
                bad_sems.add(u.id)

    def fifo_of(i):
        si = i.sync_info
        eng = str(getattr(i, "engine", None))
        if type(i).__name__ == "InstDMACopy" and si is not None:
            for u in si.on_update:
                if u.sync_type == "semaphore" and u.update_mode in UPD:
                    return ("q", u.id)
        return ("e", eng)

    cum: dict = {}
    event: dict = {}
    fifo_pred: dict = {}
    last_in_fifo: dict = {}
    metas = []
    for idx, i in enumerate(insts):
        si = getattr(i, "sync_info", None)
        f = fifo_of(i)
        fifo_pred[idx] = last_in_fifo.get(f)
        last_in_fifo[f] = idx
        ups = []
        if si is not None:
            for u in si.on_update:
                if u.sync_type == "semaphore" and u.update_mode in UPD:
                    cum[u.id] = cum.get(u.id, 0) + u.update_value
                    event[(u.id, cum[u.id])] = idx
                    ups.append((u.id, cum[u.id]))
        metas.append((si, ups))

    def resolve(sem, k):
        v = k
        while (sem, v) not in event:
            v += 1
            if v > cum.get(sem, 0):
                return None
        return event[(sem, v)]

    cvc: list = [None] * len(insts)

    def get_cvc(idx):
        if cvc[idx] is not None:
            return cvc[idx]
        stack = [idx]
        while stack:
            j = stack[-1]
            if cvc[j] is not None:
                stack.pop()
                continue
            si, ups = metas[j]
            deps = []
            p = fifo_pred[j]
            if p is not None:
                deps.append(p)
            if si is not None:
                for w in si.on_wait:
                    if (
                        w.sync_type == "semaphore"
                        and w.wait_mode == "sem-ge-imm"
                        and w.id not in bad_sems
                    ):
                        e = resolve(w.id, w.wait_value)
                        if e is not None and e != j:
                            deps.append(e)
            pending = [d for d in deps if cvc[d] is None]
            if pending:
                stack.extend(pending)
                continue
            stack.pop()
            vc: dict = {}
            for d in deps:
                for s, v in cvc[d].items():
                    if vc.get(s, 0) < v:
                        vc[s] = v
            if si is not None:
                for w in si.on_wait:
                    if (
                        w.sync_type == "semaphore"
                        and w.wait_mode == "sem-ge-imm"
                        and w.id not in bad_sems
                    ):
                        if vc.get(w.id, 0) < w.wait_value:
                            vc[w.id] = w.wait_value
            for s, v in ups:
                if vc.get(s, 0) < v:
                    vc[s] = v
            cvc[j] = vc
        return cvc[idx]

    for idx, i in enumerate(insts):
        si, _ups = metas[idx]
        if si is None or not si.on_wait:
            continue
        base: dict = {}
        p = fifo_pred[idx]
        if p is not None:
            base = dict(get_cvc(p))
        sem_waits = [
            w
            for w in si.on_wait
            if w.sync_type == "semaphore"
            and w.wait_mode == "sem-ge-imm"
            and w.id not in bad_sems
        ]
        other = [w for w in si.on_wait if w not in sem_waits]

        def strength(w):
            e = resolve(w.id, w.wait_value)
            return len(get_cvc(e)) if e is not None else 0

        sem_waits.sort(key=strength, reverse=True)

        def wait_cvc(w):
            e = resolve(w.id, w.wait_value)
            vc = dict(get_cvc(e)) if e is not None else {}
            if vc.get(w.id, 0) < w.wait_value:
                vc[w.id] = w.wait_value
            return vc

        kept = sem_waits[:]
        changed = True
        while changed:
            changed = False
            for w in kept:
                cover = dict(base)
                for w2 in kept:
                    if w2 is w:
                        continue
                    for s, v in wait_cvc(w2).items():
                        if cover.get(s, 0) < v:
                            cover[s] = v
                if cover.get(w.id, 0) >= w.wait_value:
                    kept.remove(w)
                    changed = True
                    break
        if len(kept) + len(other) != len(si.on_wait):
            si.on_wait = other + kept


def _tt_segs(g: int) -> list:
    # TT granularity: fine for the ramp groups, big (fewer DVE ops)
    # mid-stream, fine again to shorten the pipeline tail
    if g == 0:
        return [8, 8, 16]
    if g == GROUPS - 1:
        return [16, 8, 4, 4]
    return [16, 16]


def _dma_segs(g: int) -> list:
    # fbc sub-DMA granularity (TT-seg-boundary aligned).  2 MB transfers
    # measured fastest on the Sync ring; one 4 MB per group measured ~15%
    # slower end-to-end.
    return _tt_segs(g)


def _build_nc(strip: bool = True) -> bass.Bass:
    nc = bass.Bass()
    wt = nc.dram_tensor("wt", [128, NCHUNK * 128], BF16, kind="ExternalInput")
    gt2 = nc.dram_tensor("gt2", [128, BLOC * D], BF16, kind="ExternalInput")
    fbc = nc.dram_tensor("fbc", [128, GROUPS * NCHUNK * N], BF16,
                         kind="ExternalInput")
    bias = nc.dram_tensor("bias", [128, 1], F32, kind="ExternalInput")
    out = nc.dram_tensor("out", [128, BLOC * D], BF16, kind="ExternalOutput")

    GSZ = NCHUNK * N            # fbc elems per group per partition
    AF = mybir.ActivationFunctionType

    with tile.TileContext(nc) as tc:
        with (
            tc.tile_pool(name="sb", bufs=1) as res,
            tc.tile_pool(name="acc", bufs=8, space="PSUM") as apool,
        ):
            fpool = ppool = opool = res
            gt2_sb = res.tile([128, BLOC * D], BF16)
            wt_sb = res.tile([128, NCHUNK * 128], BF16)
            bias_sb = res.tile([128, 1], F32)
            scr = res.tile([128, 1], BF16)

            nc.scalar.dma_start(out=bias_sb[:], in_=bias[:])
            nc.scalar.activation(bias_sb[0:1, 0:1], bias_sb[0:1, 0:1], AF.Copy)

            fbc_tiles = {}
            rr = [0]

            def issue_fbc(g, seg_range=None):
                if g in fbc_tiles:
                    t = fbc_tiles[g]
                else:
                    t = fpool.tile([128, NCHUNK, N], BF16, tag="fbc", bufs=3)
                    fbc_tiles[g] = t
                segs = _dma_segs(g)
                c0 = sum(segs[:seg_range.start]) if seg_range else 0
                for sg in (segs[seg_range] if seg_range else segs):
                    # all fbc on the Sync HWDGE ring.  Measured alternatives:
                    # scalar-ring alternation 13% slower (issue-side WAR
                    # waits stall the ACT sequencer's output chain), GPSIMD
                    # SWDGE alternation 27% slower (Q7 descriptor emission
                    # overhead per transfer).
                    nc.sync.dma_start(
                        out=t[:, c0:c0 + sg, :],
                        in_=fbc[:, g * GSZ + c0 * N:g * GSZ + (c0 + sg) * N],
                    )
                    c0 += sg

            # ramp: group-0's gt2 slice, then wt and the gt2 remainder, then
            # the fbc stream — all on the Sync ring in FIFO order
            nc.sync.dma_start(out=gt2_sb[:, ts(0, N)], in_=gt2[:, ts(0, N)])
            nc.sync.dma_start(out=wt_sb[:], in_=wt[:])
            nc.sync.dma_start(out=gt2_sb[:, N:], in_=gt2[:, N:])
            nc.vector.tensor_copy(gt2_sb[0:1, 0:1], gt2_sb[0:1, 0:1])
            nc.vector.tensor_copy(wt_sb[0:1, 0:1], wt_sb[0:1, 0:1])
            nc.vector.tensor_copy(gt2_sb[0:1, N:N + 1], gt2_sb[0:1, N:N + 1])

            issue_fbc(0)
            issue_fbc(1)

            for g in range(GROUPS):
                fbc_t = fbc_tiles.pop(g)
                if g + 2 < GROUPS:
                    issue_fbc(g + 2)
                acc = apool.tile([128, N], F32, tag="acc")
                gblk = gt2_sb[:, ts(g, N)]
                bounds = set()
                acc_b = 0
                for sg in _dma_segs(g):
                    bounds.add(acc_b)
                    acc_b += sg
                c0 = 0
                for sg in _tt_segs(g):
                    if c0 in bounds:
                        # touch: sub-DMA completion sem onto the DVE clock
                        nc.vector.tensor_copy(fbc_t[0:1, c0, 0:1],
                                              fbc_t[0:1, c0, 0:1])
                    p = ppool.tile([128, sg, N], BF16, tag=f"p{sg}",
                                   bufs=3 if sg >= 16 else 2)
                    gview = gblk[:, None, :].to_broadcast((128, sg, N))
                    nc.vector.tensor_mul(p[:], gview,
                                         fbc_t[:, c0:c0 + sg, :])
                    for u in range(sg):
                        c = c0 + u
                        nc.tensor.matmul(acc[:], wt_sb[:, ts(c, 128)],
                                         p[:, u, :], start=(c == 0),
                                         stop=(c == NCHUNK - 1))
                    c0 += sg

                osb = opool.tile([128, N], BF16, tag="osb", bufs=8)
                nc.scalar.activation(osb[:], acc[:], AF.Identity,
                                     bias=bias_sb[:, 0:1], scale=1.0)
                nc.scalar.dma_start(out=out[:, ts(g, N)], in_=osb[:])
                # WAR consumer: pulls the out-DMA's queue-sem onto the ACT
                # clock so the kernel-tail drain needs only engine waits
                nc.scalar.activation(osb[0:1, 0:1], osb[0:1, 0:1], AF.Copy)
                last_osb = osb

            # fold the ACT-final clock onto DVE so the kernel-exit Drain
            # (a 1-wait CTRL struct) needs only the DVE sem
            nc.vector.tensor_copy(scr[0:1, 0:1], last_osb[0:1, 0:1])

    if strip:
        _strip_self_waits(nc)
    return nc


def _get_nc() -> bass.Bass:
    if "nc" not in _CACHE:
        _CACHE["nc"] = _build_nc()
    return _CACHE["nc"]


def _prep_core_inputs(feature, base, W, b, ci):
    bf16 = mybir.dt.np(BF16)
    bsl = slice(ci * BLOC, (ci + 1) * BLOC)
    F = np.ascontiguousarray(feature[bsl], np.float32)  # (128, 64, 32)
    G = np.ascontiguousarray(base[bsl], np.float32)     # (128, 64, 32)

    Gt = np.transpose(G, (1, 0, 2))                      # (j, b, d)
    gt2 = np.concatenate([Gt, Gt], 0).reshape(128, BLOC * D)

    # wt[p, c, o] = W[o, 128c + p]
    wt = np.transpose(W.reshape(O, NCHUNK, 128), (2, 1, 0)).reshape(
        128, NCHUNK * 128)

    # fbc[p=(dup,64), g, c, n] = F-transposed[i=2c+dup, g, n] replicated over
    # the 64 j-partitions of each dup-half
    Ft = np.transpose(F, (1, 0, 2)).reshape(HK, GROUPS, N)   # [i, g, n]
    A = Ft.reshape(NCHUNK, 2, GROUPS, N).astype(bf16)        # [c, dup, g, n]
    fbc = np.broadcast_to(
        A.transpose(1, 2, 0, 3)[:, None], (2, 64, GROUPS, NCHUNK, N)
    )
    fbc = np.ascontiguousarray(fbc).reshape(128, GROUPS * NCHUNK * N)

    return {
        "wt": np.ascontiguousarray(wt.astype(bf16)),
        "gt2": np.ascontiguousarray(gt2.astype(bf16)),
        "fbc": fbc,
        "bias": np.ascontiguousarray(b, np.float32).reshape(128, 1),
    }


def run(feature, base, W, b, **spmd_kwargs):
    nc = _get_nc()
    in_maps = [_prep_core_inputs(feature, base, W, b, ci) for ci in range(NCORES)]
    res = run_bass_kernel_spmd(nc, in_maps, list(range(NCORES)), **spmd_kwargs)
    outs = []
    for ci in range(NCORES):
        o = res.results[ci]["out"].astype(np.float32).reshape(O, BLOC, D)
        outs.append(np.transpose(o, (1, 0, 2)))
    full = np.concatenate(outs, 0)
    return full, res


def kernel(feature, base, W, b):
    full, _ = run(feature, base, W, b)
    return full
